# revision 7
# baseline (speedup 1.0000x reference)
"""Capsule dynamic-routing kernel v4 for Trainium2 (Bass/Tile), 8 NeuronCores.

Data-parallel over batch (B=64 -> 8 batches/core, 4 pairs of 2). W is tiny
and folded into per-iteration stationary operands; no collectives.

Math (see the b/G decomposition below): u_hat is never materialized.
    b_i[n,k] = <Wo[:,k], u[n,:]>,  Wo = W @ Obd(o_acc)
    G[k,e]   = sum_n c[k,n] u[n,e]
    s[k,:]   = G[k,:] @ W-block(k); out = squash(s)
The b-path tolerates fp8 u (per-n-independent noise washes out of the
routing sums, ~3e-3 final rel_err); the G-path needs bf16 (its noise is
coherent in the output).

v4 structure: the routing is batch-local, so each 2-batch PAIR runs its
entire 3-iteration routing independently. DMA is issued pair-major
(un_p, ut_p, un_{p+1}, ...) and each pair's compute starts as soon as its
own data lands, overlapping the remaining pairs' DMA. The serial tail
after the last byte arrives is one pair's work, not a full sweep.

Residents per pair: un (bf16, [128n x 2b*64e] chunks; G-pass stationary)
and ut (fp8e4, [2b*64e x n]; b-pass stationary, also halves LDWEIGHTS).

Softmax: exp on Scalar (bf16 out), group-reduce + reciprocal on Vector,
the c = e*zr scale alternates Vector/GpSimd so no one engine gates.
"""

import numpy as np
from contextlib import ExitStack

import ml_dtypes

import concourse.bass as bass
import concourse.bacc as bacc
import concourse.tile as tile
import concourse.mybir as mybir
from concourse.bass_utils import run_bass_kernel_spmd

dt = mybir.dt
AFT = mybir.ActivationFunctionType
AXT = mybir.AxisListType

B, N_FULL, D = 64, 8192, 64
K, DCAP, KD = 16, 16, 256
NCORES = 8
NB = 8            # batches per core
NP = 4            # batch pairs per core
ROUTINGS = 3
EPS = 1e-7
CHUNK = 128       # n per contraction chunk
SUP = 16          # chunks per softmax super-chunk
TCOLS = 2048      # free columns per resident DMA sub-tile

U_DT = dt.bfloat16
U_NP = ml_dtypes.bfloat16
U8_DT = dt.float8e4
U8_NP = ml_dtypes.float8_e4m3


def build_program(n=N_FULL, reps=1, ablate=(), sup=SUP):
    assert n % CHUNK == 0
    nch = n // CHUNK                  # chunks per pair
    sup = min(sup, nch)
    assert nch % sup == 0
    nsup = nch // sup                 # softmax units per pair per pass
    tcols = min(TCOLS, n)
    ntile = n // tcols                # resident sub-tiles per pair (each fmt)
    cpt = tcols // CHUNK              # chunks per sub-tile
    f32 = dt.float32

    nc = bacc.Bacc("TRN2", target_bir_lowering=False, debug=False)

    un_d = nc.dram_tensor("un", [NP, ntile, 128, tcols], U_DT,
                          kind="ExternalInput").ap()
    ut_d = nc.dram_tensor("ut", [NP, ntile, 128, tcols], U8_DT,
                          kind="ExternalInput").ap()
    wt_d = nc.dram_tensor("wt", [2, 128, D], U_DT, kind="ExternalInput").ap()
    wsb_d = nc.dram_tensor("wsb", [128, KD], U_DT, kind="ExternalInput").ap()
    mask_d = nc.dram_tensor("mask", [128, KD], f32, kind="ExternalInput").ap()
    ident_d = nc.dram_tensor("ident", [128, 128], f32, kind="ExternalInput").ap()
    out_d = nc.dram_tensor("out", [128, KD], f32, kind="ExternalOutput").ap()

    with tile.TileContext(nc) as tc, ExitStack() as ctx:
        consts = ctx.enter_context(tc.tile_pool(name="consts", bufs=1))
        resident = ctx.enter_context(tc.tile_pool(name="resident", bufs=1))
        work = ctx.enter_context(tc.tile_pool(name="work", bufs=1))
        c_pool = ctx.enter_context(tc.tile_pool(name="cpool", bufs=4))
        e_pool = ctx.enter_context(tc.tile_pool(name="epool", bufs=4))
        z_pool = ctx.enter_context(tc.tile_pool(name="zpool", bufs=8))
        ps_bb = ctx.enter_context(tc.tile_pool(name="psbb", bufs=3, space="PSUM"))
        ps_gt = ctx.enter_context(tc.tile_pool(name="psgt", bufs=2, space="PSUM"))
        ps_fin = ctx.enter_context(tc.tile_pool(name="psfin", bufs=2, space="PSUM"))

        # ---- constants ----
        wt_t = consts.tile([128, 2 * D], U_DT, tag="wt", name="wt")
        for h in range(2):
            nc.sync.dma_start(out=wt_t[:, h * D:(h + 1) * D], in_=wt_d[h])
        wsb_t = consts.tile([128, KD], U_DT, tag="wsb", name="wsb")
        nc.sync.dma_start(out=wsb_t[:, :], in_=wsb_d[:, :])
        mask_t = consts.tile([128, KD], f32, tag="mask", name="mask")
        nc.sync.dma_start(out=mask_t[:, :], in_=mask_d[:, :])
        ident_t = consts.tile([128, 128], f32, tag="ident", name="ident")
        nc.sync.dma_start(out=ident_t[:, :], in_=ident_d[:, :])
        cu_t = consts.tile([128, 32], U_DT, tag="cu", name="cu")
        nc.vector.memset(cu_t[:, :], 1.0 / K)
        eps_t = consts.tile([128, 1], f32, tag="eps", name="eps")
        nc.vector.memset(eps_t[:, :], EPS)

        # ---- residents (per pair) ----
        un_t = [[resident.tile([128, tcols], U_DT, tag=f"un{p}_{q}",
                               name=f"un{p}_{q}") for q in range(ntile)]
                for p in range(NP)]
        ut_t = [[resident.tile([128, tcols], U8_DT, tag=f"ut{p}_{q}",
                               name=f"ut{p}_{q}") for q in range(ntile)]
                for p in range(NP)]

        def un_chunk(p, j):
            return un_t[p][j // cpt][:, (j % cpt) * CHUNK:(j % cpt + 1) * CHUNK]

        def ut_chunk(p, j):
            return ut_t[p][j // cpt][:, (j % cpt) * CHUNK:(j % cpt + 1) * CHUNK]

        # ---- per-pair work tiles (separate tiles -> no cross-pair deps) ----
        o_acc = [work.tile([32, KD], f32, tag=f"oacc{p}", name=f"oacc{p}")
                 for p in range(NP)]
        sm = [work.tile([32, KD], f32, tag=f"sm{p}", name=f"sm{p}")
              for p in range(NP)]
        sq = [work.tile([32, KD], f32, tag=f"sq{p}", name=f"sq{p}")
              for p in range(NP)]
        o_fin = [work.tile([32, KD], f32, tag=f"ofin{p}", name=f"ofin{p}")
                 for p in range(NP)]
        t1_sb = [work.tile([128, 32], U_DT, tag=f"t1_{p}", name=f"t1_{p}")
                 for p in range(NP)]
        t2_sb = [work.tile([128, 32], U_DT, tag=f"t2_{p}", name=f"t2_{p}")
                 for p in range(NP)]
        wop = [work.tile([128, 32], U_DT, tag=f"wop{p}", name=f"wop{p}")
               for p in range(NP)]
        gt_sb = [work.tile([128, 32], U_DT, tag=f"gts{p}", name=f"gts{p}")
                 for p in range(NP)]
        s2 = [work.tile([32, 1], f32, tag=f"s2_{p}", name=f"s2_{p}")
              for p in range(NP)]
        sca = [work.tile([32, 1], f32, tag=f"sca{p}", name=f"sca{p}")
               for p in range(NP)]
        scb = [work.tile([32, 1], f32, tag=f"scb{p}", name=f"scb{p}")
               for p in range(NP)]
        sce = [work.tile([32, 1], f32, tag=f"sce{p}", name=f"sce{p}")
               for p in range(NP)]
        eps32 = consts.tile([32, 1], f32, tag="eps32", name="eps32")
        nc.vector.memset(eps32[:, :], EPS)

        # cross-batch blocks of gt_sb / wop stay zero for the whole kernel
        for p in range(NP):
            nc.vector.memset(gt_sb[p][0:64, 16:32], 0.0)
            nc.vector.memset(gt_sb[p][64:128, 0:16], 0.0)
            nc.vector.memset(wop[p][0:64, 16:32], 0.0)
            nc.vector.memset(wop[p][64:128, 0:16], 0.0)

        mulsel = [0]

        def pass_pair(p, it):
            """One routing pass for pair p: (b-pass+softmax if it>0) + G."""
            gt = ps_gt.tile([128, 32], f32, tag="gt", name=f"gt{p}_{it}",
                            padded_shape=[128, 512])
            for s in range(nsup):
                if it == 0 or "nobb" in ablate:
                    def c_src(rel):
                        return cu_t[:, :]
                else:
                    bb = ps_bb.tile([128, sup * 32], f32, tag="bb", name="bb",
                                    padded_shape=[128, 512])
                    for rel in range(sup):
                        j = s * sup + rel
                        nc.tensor.matmul(
                            bb[:, rel * 32:(rel + 1) * 32],
                            lhsT=ut_chunk(p, j), rhs=wop[p][:, :],
                            start=(rel == 0), stop=(rel == sup - 1))
                    e_t = e_pool.tile([128, sup * 32], U_DT, tag="e", name="e")
                    nc.scalar.activation(e_t[:, :], bb[:, :], AFT.Exp)
                    z_t = z_pool.tile([128, sup * 2], f32, tag="z", name="z")
                    nc.vector.reduce_sum(
                        z_t[:, :].rearrange("p (a b) -> p a b", b=2),
                        e_t[:, :].rearrange("p (a b c) -> p a b c", b=2, c=K),
                        axis=AXT.X)
                    zr_t = z_pool.tile([128, sup * 2], f32, tag="zr", name="zr")
                    nc.vector.reciprocal(zr_t[:, :], z_t[:, :])
                    c_t = c_pool.tile([128, sup * 32], U_DT, tag="c", name="c")
                    eng = nc.vector if mulsel[0] % 2 == 0 else nc.gpsimd
                    mulsel[0] += 1
                    eng.tensor_mul(
                        c_t[:, :].rearrange("p (a b c) -> p a b c", b=2, c=K),
                        e_t[:, :].rearrange("p (a b c) -> p a b c", b=2, c=K),
                        zr_t[:, :].rearrange("p (a b) -> p a b", b=2)
                            .broadcast_to([128, sup, 2, K]))

                    def c_src(rel, c_t=c_t):
                        return c_t[:, rel * 32:(rel + 1) * 32]
                for rel in range(sup):
                    j = s * sup + rel
                    nc.tensor.matmul(
                        gt[:, :],
                        lhsT=un_chunk(p, j), rhs=c_src(rel),
                        start=(j == 0), stop=(j == nch - 1))
            return gt

        def fin_pair(p, it, gt):
            """gt -> s -> mask -> squash -> o; update wop[p] (if not last)."""
            r0, r1 = 32 * p, 32 * p + 32
            nc.vector.tensor_copy(gt_sb[p][0:64, 0:16], gt[0:64, 0:16])
            nc.vector.tensor_copy(gt_sb[p][64:128, 16:32], gt[64:128, 16:32])
            sf = ps_fin.tile([32, KD], f32, tag="fin", name=f"sf{p}_{it}",
                             padded_shape=[32, 512])
            nc.tensor.matmul(sf[:, :], lhsT=gt_sb[p][:, :], rhs=wsb_t[:, :],
                             start=True, stop=True)
            nc.vector.tensor_mul(sm[p][:, :], sf[:, :], mask_t[r0:r1, :])
            nc.scalar.activation(sq[p][:, :], sm[p][:, :], AFT.Square,
                                 accum_out=s2[p][:, :])
            nc.vector.tensor_scalar_add(sca[p][:, :], s2[p][:, :], 1.0)
            nc.vector.reciprocal(sca[p][:, :], sca[p][:, :])
            nc.scalar.activation(scb[p][:, :], s2[p][:, :], AFT.Sqrt,
                                 bias=eps32[:, :])
            nc.vector.reciprocal(scb[p][:, :], scb[p][:, :])
            nc.vector.tensor_mul(sce[p][:, :], sca[p][:, :], scb[p][:, :])
            nc.vector.tensor_mul(sce[p][:, :], sce[p][:, :], s2[p][:, :])
            if it == ROUTINGS - 1:
                nc.vector.tensor_scalar_mul(o_fin[p][:, :], sm[p][:, :],
                                            sce[p][:, :])
                nc.sync.dma_start(out=out_d[r0:r1, :], in_=o_fin[p][:, :])
                return
            if it == 1:
                nc.vector.tensor_scalar_mul(o_fin[p][:, :], sm[p][:, :],
                                            sce[p][:, :])
                nc.vector.tensor_add(o_acc[p][:, :], o_acc[p][:, :],
                                     o_fin[p][:, :])
            else:
                nc.vector.tensor_scalar_mul(o_acc[p][:, :], sm[p][:, :],
                                            sce[p][:, :])
            # Obd halves of this pair's o_acc rows -> t1/t2 columns
            for h, t_sb in ((0, t1_sb[p]), (1, t2_sb[p])):
                tp = ps_fin.tile([128, 32], f32, tag="fin", name=f"tp{p}{h}{it}",
                                 padded_shape=[128, 512])
                nc.tensor.transpose(tp[:, :],
                                    o_acc[p][:, h * 128:(h + 1) * 128],
                                    ident_t[0:32, 0:32])
                nc.vector.tensor_copy(t_sb[:, :], tp[:, :])
            # Wo for the pair's 2 batches: accumulate over the two W.T halves
            wo = ps_fin.tile([64, 2 * K], f32, tag="fin", name=f"wo{p}_{it}",
                             padded_shape=[64, 512])
            for h2 in range(2):
                for hb in range(2):
                    nc.tensor.matmul(
                        wo[:, hb * K:(hb + 1) * K],
                        lhsT=wt_t[:, h2 * D:(h2 + 1) * D],
                        rhs=(t1_sb[p], t2_sb[p])[h2][:, hb * K:(hb + 1) * K],
                        start=(h2 == 0 and hb == 0),
                        stop=(h2 == 1 and hb == 1))
            for hb in range(2):
                nc.vector.tensor_copy(
                    wop[p][64 * hb:64 * hb + 64, 16 * hb:16 * hb + 16],
                    wo[:, hb * K:(hb + 1) * K])

        for rep in range(reps):
            if "nodma" not in ablate:
                for p in range(NP):
                    for q in range(ntile):
                        nc.sync.dma_start(out=un_t[p][q][:, :], in_=un_d[p, q])
                    for q in range(ntile):
                        nc.sync.dma_start(out=ut_t[p][q][:, :], in_=ut_d[p, q])
            elif rep == 0:
                for p in range(NP):
                    for q in range(ntile):
                        nc.vector.memset(un_t[p][q][:, 0:2], 0.0)
                        nc.vector.memset(ut_t[p][q][:, 0:2], 0.0)
            if "nocompute" not in ablate:
                for p in range(NP):
                    for it in range(ROUTINGS):
                        gt = pass_pair(p, it)
                        fin_pair(p, it, gt)
            else:
                for p in range(NP):
                    nc.vector.memset(o_fin[p][:, :], 0.0)
                    nc.sync.dma_start(out=out_d[32 * p:32 * p + 32, :],
                                      in_=o_fin[p][:, :])
            if rep < reps - 1:
                tc.strict_bb_all_engine_barrier()

    nc.compile()
    return nc


def host_inputs(u_shard, W):
    """Per-core DRAM inputs from an (8, N, 64) f32 batch shard."""
    n = u_shard.shape[1]
    tcols = min(TCOLS, n)
    ntile = n // tcols
    cpt = tcols // CHUNK
    # ut tile q of pair p: [(h,e), n-slice]  (e-on-partitions, fp8)
    ut = np.ascontiguousarray(
        u_shard.reshape(NP, 2, ntile, tcols, D)
        .transpose(0, 2, 1, 4, 3).reshape(NP, ntile, 128, tcols)
    ).astype(U8_NP)
    # un tile q of pair p: [n-within-chunk, (chunk-rel, h, e)]
    un = np.ascontiguousarray(
        u_shard.reshape(NP, 2, ntile, cpt, CHUNK, D)
        .transpose(0, 2, 4, 3, 1, 5).reshape(NP, ntile, 128, tcols)
    ).astype(U_NP)
    return {"ut": ut, "un": un}


def host_consts(W):
    Wf = np.asarray(W, np.float32)
    wt = np.ascontiguousarray(Wf.T.reshape(2, 128, D)).astype(U_NP)
    wsb = np.ascontiguousarray(np.concatenate([Wf, Wf], 0)).astype(U_NP)
    base = np.kron(np.eye(K, dtype=np.float32), np.ones((1, DCAP), np.float32))
    mask = np.ascontiguousarray(np.tile(base, (NB, 1)))
    ident = np.eye(128, dtype=np.float32)
    return {"wt": wt, "wsb": wsb, "mask": mask, "ident": ident}


def extract_output(res_out):
    """(128, 256) masked f32 -> (8, 16, 16) squashed capsule outputs."""
    ar = np.arange(K)
    return res_out.reshape(NB, K, K, DCAP)[:, ar, ar, :]


_PROG_CACHE = {}


def _get_prog(n=N_FULL, reps=1):
    key = (n, reps)
    if key not in _PROG_CACHE:
        _PROG_CACHE[key] = build_program(n, reps)
    return _PROG_CACHE[key]


def kernel(u_vecs, W):
    u = np.ascontiguousarray(np.asarray(u_vecs, np.float32))
    assert u.shape == (B, N_FULL, D)
    nc = _get_prog()
    consts = host_consts(W)
    in_maps = [dict(consts, **host_inputs(u[c * NB:(c + 1) * NB], W))
               for c in range(NCORES)]
    res = run_bass_kernel_spmd(nc, in_maps, core_ids=list(range(NCORES)))
    return np.concatenate(
        [extract_output(res.results[c]["out"]) for c in range(NCORES)], axis=0
    ).astype(np.float32)


# revision 11
# speedup vs baseline: 2.5137x; 2.5137x over previous
"""Capsule dynamic-routing kernel v4 for Trainium2 (Bass/Tile), 8 NeuronCores.

Data-parallel over batch (B=64 -> 8 batches/core, 4 pairs of 2). W is tiny
and folded into per-iteration stationary operands; no collectives.

Math (see the b/G decomposition below): u_hat is never materialized.
    b_i[n,k] = <Wo[:,k], u[n,:]>,  Wo = W @ Obd(o_acc)
    G[k,e]   = sum_n c[k,n] u[n,e]
    s[k,:]   = G[k,:] @ W-block(k); out = squash(s)
The b-path tolerates fp8 u (per-n-independent noise washes out of the
routing sums, ~3e-3 final rel_err); the G-path needs bf16 (its noise is
coherent in the output).

v4 structure: the routing is batch-local, so each 2-batch PAIR runs its
entire 3-iteration routing independently. DMA is issued pair-major
(un_p, ut_p, un_{p+1}, ...) and each pair's compute starts as soon as its
own data lands, overlapping the remaining pairs' DMA. The serial tail
after the last byte arrives is one pair's work, not a full sweep.

Residents per pair: un (bf16, [128n x 2b*64e] chunks; G-pass stationary)
and ut (fp8e4, [2b*64e x n]; b-pass stationary, also halves LDWEIGHTS).

Softmax: exp on Scalar (bf16 out), group-reduce + reciprocal on Vector,
the c = e*zr scale alternates Vector/GpSimd so no one engine gates.
"""

import numpy as np
from contextlib import ExitStack

import ml_dtypes

import concourse.bass as bass
import concourse.bacc as bacc
import concourse.tile as tile
import concourse.mybir as mybir
import concourse.hw_specs as hw_specs
from concourse.bass_utils import run_bass_kernel_spmd

# The act-table chooser loads, for each activation, the FIRST table set
# containing its function. Exp alone -> "exp_and_others", Ln alone ->
# "natural_log"; a kernel using both ping-pongs between sets at ~2.7us per
# ACT_TABLE_LOAD. Hide exp/ln from the single-function sets so every
# activation resolves to the combined "natural_log_exp_and_others" set
# (its real act_info.json index is preserved, so the load is valid) and
# the whole program needs exactly one table load.
_ORIG_GAT = hw_specs.get_activation_tables


def _patched_gat(arch):
    tabs = dict(_ORIG_GAT(arch))
    combined = "natural_log_exp_and_others"
    if combined in tabs:
        hide = {mybir.ActivationFunctionType.Exp,
                mybir.ActivationFunctionType.Ln,
                mybir.ActivationFunctionType.Square}
        for name in list(tabs):
            if name != combined:
                tabs[name] = set(tabs[name]) - hide
    return tabs


hw_specs.get_activation_tables = _patched_gat
bacc.get_activation_tables = _patched_gat

dt = mybir.dt
AFT = mybir.ActivationFunctionType
AXT = mybir.AxisListType

B, N_FULL, D = 64, 8192, 64
K, DCAP, KD = 16, 16, 256
NCORES = 8
NB = 8            # batches per core
NP = 4            # batch pairs per core
ROUTINGS = 3
EPS = 1e-7
CHUNK = 128       # n per contraction chunk
SUP = 16          # chunks per softmax super-chunk
TCOLS = 2048      # free columns per resident DMA sub-tile

U_DT = dt.bfloat16
U_NP = ml_dtypes.bfloat16
U8_DT = dt.float8e4
U8_NP = ml_dtypes.float8_e4m3


def build_program(n=N_FULL, reps=1, ablate=(), sup=SUP):
    assert n % CHUNK == 0
    nch = n // CHUNK                  # chunks per pair
    sup = min(sup, nch)
    assert nch % sup == 0
    nsup = nch // sup                 # softmax units per pair per pass
    tcols = min(TCOLS, n)
    ntile = n // tcols                # resident sub-tiles per pair (each fmt)
    cpt = tcols // CHUNK              # chunks per sub-tile
    f32 = dt.float32

    nc = bacc.Bacc("TRN2", target_bir_lowering=False, debug=False)

    un_d = nc.dram_tensor("un", [NP, ntile, 128, tcols], U_DT,
                          kind="ExternalInput").ap()
    ut_d = nc.dram_tensor("ut", [NP, ntile, 128, tcols], U8_DT,
                          kind="ExternalInput").ap()
    wt_d = nc.dram_tensor("wt", [2, 128, D], U_DT, kind="ExternalInput").ap()
    wsb_d = nc.dram_tensor("wsb", [128, KD], U_DT, kind="ExternalInput").ap()
    mask_d = nc.dram_tensor("mask", [128, KD], f32, kind="ExternalInput").ap()
    ident_d = nc.dram_tensor("ident", [128, 128], f32, kind="ExternalInput").ap()
    out_d = nc.dram_tensor("out", [128, KD], f32, kind="ExternalOutput").ap()

    with tile.TileContext(nc) as tc, ExitStack() as ctx:
        consts = ctx.enter_context(tc.tile_pool(name="consts", bufs=1))
        resident = ctx.enter_context(tc.tile_pool(name="resident", bufs=1))
        work = ctx.enter_context(tc.tile_pool(name="work", bufs=1))
        c_pool = ctx.enter_context(tc.tile_pool(name="cpool", bufs=4))
        e_pool = ctx.enter_context(tc.tile_pool(name="epool", bufs=4))
        z_pool = ctx.enter_context(tc.tile_pool(name="zpool", bufs=8))
        ps_bb = ctx.enter_context(tc.tile_pool(name="psbb", bufs=3, space="PSUM"))
        ps_gt = ctx.enter_context(tc.tile_pool(name="psgt", bufs=2, space="PSUM"))
        ps_fin = ctx.enter_context(tc.tile_pool(name="psfin", bufs=2, space="PSUM"))

        # ---- constants ----
        wt_t = consts.tile([128, 2 * D], U_DT, tag="wt", name="wt")
        for h in range(2):
            nc.sync.dma_start(out=wt_t[:, h * D:(h + 1) * D], in_=wt_d[h])
        wsb_t = consts.tile([128, KD], U_DT, tag="wsb", name="wsb")
        nc.sync.dma_start(out=wsb_t[:, :], in_=wsb_d[:, :])
        mask_t = consts.tile([128, KD], f32, tag="mask", name="mask")
        nc.sync.dma_start(out=mask_t[:, :], in_=mask_d[:, :])
        ident_t = consts.tile([128, 128], f32, tag="ident", name="ident")
        nc.sync.dma_start(out=ident_t[:, :], in_=ident_d[:, :])
        cu_t = consts.tile([128, 32], U_DT, tag="cu", name="cu")
        nc.vector.memset(cu_t[:, :], 1.0 / K)
        eps_t = consts.tile([128, 1], f32, tag="eps", name="eps")
        nc.vector.memset(eps_t[:, :], EPS)

        # ---- residents (per pair) ----
        un_t = [[resident.tile([128, tcols], U_DT, tag=f"un{p}_{q}",
                               name=f"un{p}_{q}") for q in range(ntile)]
                for p in range(NP)]
        ut_t = [[resident.tile([128, tcols], U8_DT, tag=f"ut{p}_{q}",
                               name=f"ut{p}_{q}") for q in range(ntile)]
                for p in range(NP)]

        def un_chunk(p, j):
            return un_t[p][j // cpt][:, (j % cpt) * CHUNK:(j % cpt + 1) * CHUNK]

        def ut_chunk(p, j):
            return ut_t[p][j // cpt][:, (j % cpt) * CHUNK:(j % cpt + 1) * CHUNK]

        # ---- per-pair work tiles (separate tiles -> no cross-pair deps) ----
        o_acc = [work.tile([32, KD], f32, tag=f"oacc{p}", name=f"oacc{p}")
                 for p in range(NP)]
        sm = [work.tile([32, KD], f32, tag=f"sm{p}", name=f"sm{p}")
              for p in range(NP)]
        sq = [work.tile([32, KD], f32, tag=f"sq{p}", name=f"sq{p}")
              for p in range(NP)]
        o_fin = [work.tile([32, KD], f32, tag=f"ofin{p}", name=f"ofin{p}")
                 for p in range(NP)]
        t1_sb = [work.tile([128, 32], U_DT, tag=f"t1_{p}", name=f"t1_{p}")
                 for p in range(NP)]
        t2_sb = [work.tile([128, 32], U_DT, tag=f"t2_{p}", name=f"t2_{p}")
                 for p in range(NP)]
        wop = [work.tile([128, 32], U_DT, tag=f"wop{p}", name=f"wop{p}")
               for p in range(NP)]
        gt_sb = [work.tile([128, 32], U_DT, tag=f"gts{p}", name=f"gts{p}")
                 for p in range(NP)]
        s2 = [work.tile([32, 1], f32, tag=f"s2_{p}", name=f"s2_{p}")
              for p in range(NP)]
        sca = [work.tile([32, 1], f32, tag=f"sca{p}", name=f"sca{p}")
               for p in range(NP)]
        scb = [work.tile([32, 1], f32, tag=f"scb{p}", name=f"scb{p}")
               for p in range(NP)]
        sce = [work.tile([32, 1], f32, tag=f"sce{p}", name=f"sce{p}")
               for p in range(NP)]
        eps32 = consts.tile([32, 1], f32, tag="eps32", name="eps32")
        nc.vector.memset(eps32[:, :], EPS)
        neghalf = consts.tile([32, 1], f32, tag="nhalf", name="nhalf")
        nc.vector.memset(neghalf[:, :], -0.5)

        # cross-batch blocks of gt_sb / wop stay zero for the whole kernel
        for p in range(NP):
            nc.vector.memset(gt_sb[p][0:64, 16:32], 0.0)
            nc.vector.memset(gt_sb[p][64:128, 0:16], 0.0)
            nc.vector.memset(wop[p][0:64, 16:32], 0.0)
            nc.vector.memset(wop[p][64:128, 0:16], 0.0)

        mulsel = [0]

        def pass_pair(p, it):
            """One routing pass for pair p: (b-pass+softmax if it>0) + G."""
            gt = ps_gt.tile([128, 32], f32, tag="gt", name=f"gt{p}_{it}",
                            padded_shape=[128, 512])
            for s in range(nsup):
                if it == 0 or "nobb" in ablate:
                    def c_src(rel):
                        return cu_t[:, :]
                else:
                    bb = ps_bb.tile([128, sup * 32], f32, tag="bb", name="bb",
                                    padded_shape=[128, 512])
                    for rel in range(sup):
                        j = s * sup + rel
                        nc.tensor.matmul(
                            bb[:, rel * 32:(rel + 1) * 32],
                            lhsT=ut_chunk(p, j), rhs=wop[p][:, :],
                            start=(rel == 0), stop=(rel == sup - 1))
                    e_t = e_pool.tile([128, sup * 32], U_DT, tag="e", name="e")
                    nc.scalar.activation(e_t[:, :], bb[:, :], AFT.Exp)
                    z_t = z_pool.tile([128, sup * 2], f32, tag="z", name="z")
                    nc.vector.reduce_sum(
                        z_t[:, :].rearrange("p (a b) -> p a b", b=2),
                        e_t[:, :].rearrange("p (a b c) -> p a b c", b=2, c=K),
                        axis=AXT.X)
                    zr_t = z_pool.tile([128, sup * 2], f32, tag="zr", name="zr")
                    nc.vector.reciprocal(zr_t[:, :], z_t[:, :])
                    c_t = c_pool.tile([128, sup * 32], U_DT, tag="c", name="c")
                    eng = nc.vector if mulsel[0] % 2 == 0 else nc.gpsimd
                    mulsel[0] += 1
                    eng.tensor_mul(
                        c_t[:, :].rearrange("p (a b c) -> p a b c", b=2, c=K),
                        e_t[:, :].rearrange("p (a b c) -> p a b c", b=2, c=K),
                        zr_t[:, :].rearrange("p (a b) -> p a b", b=2)
                            .broadcast_to([128, sup, 2, K]))

                    def c_src(rel, c_t=c_t):
                        return c_t[:, rel * 32:(rel + 1) * 32]
                for rel in range(sup):
                    j = s * sup + rel
                    nc.tensor.matmul(
                        gt[:, :],
                        lhsT=un_chunk(p, j), rhs=c_src(rel),
                        start=(j == 0), stop=(j == nch - 1))
            return gt

        def fin_pair(p, it, gt):
            """gt -> s -> mask -> squash -> o; update wop[p] (if not last)."""
            r0, r1 = 32 * p, 32 * p + 32
            nc.vector.tensor_copy(gt_sb[p][0:64, 0:16], gt[0:64, 0:16])
            nc.vector.tensor_copy(gt_sb[p][64:128, 16:32], gt[64:128, 16:32])
            sf = ps_fin.tile([32, KD], f32, tag="fin", name=f"sf{p}_{it}",
                             padded_shape=[32, 512])
            nc.tensor.matmul(sf[:, :], lhsT=gt_sb[p][:, :], rhs=wsb_t[:, :],
                             start=True, stop=True)
            nc.vector.tensor_mul(sm[p][:, :], sf[:, :], mask_t[r0:r1, :])
            nc.scalar.activation(sq[p][:, :], sm[p][:, :], AFT.Square,
                                 accum_out=s2[p][:, :])
            # rsqrt(s2+eps) = exp(-0.5*ln(s2+eps)): keeps every scalar op in
            # the natural_log_exp_and_others table set -> no ACT_TABLE_LOADs
            # (~2.7us each) between the fins and the softmax exps.
            nc.scalar.activation(scb[p][:, :], s2[p][:, :], AFT.Ln,
                                 bias=eps32[:, :])
            nc.scalar.activation(scb[p][:, :], scb[p][:, :], AFT.Exp,
                                 scale=neghalf[:, :])
            nc.vector.tensor_scalar_add(sca[p][:, :], s2[p][:, :], 1.0)
            nc.vector.reciprocal(sca[p][:, :], sca[p][:, :])
            nc.vector.tensor_mul(sce[p][:, :], sca[p][:, :], scb[p][:, :])
            nc.vector.tensor_mul(sce[p][:, :], sce[p][:, :], s2[p][:, :])
            if it == ROUTINGS - 1:
                nc.vector.tensor_scalar_mul(o_fin[p][:, :], sm[p][:, :],
                                            sce[p][:, :])
                nc.sync.dma_start(out=out_d[r0:r1, :], in_=o_fin[p][:, :])
                return
            if it == 1:
                nc.vector.tensor_scalar_mul(o_fin[p][:, :], sm[p][:, :],
                                            sce[p][:, :])
                nc.vector.tensor_add(o_acc[p][:, :], o_acc[p][:, :],
                                     o_fin[p][:, :])
            else:
                nc.vector.tensor_scalar_mul(o_acc[p][:, :], sm[p][:, :],
                                            sce[p][:, :])
            # Obd halves of this pair's o_acc rows -> t1/t2 columns
            for h, t_sb in ((0, t1_sb[p]), (1, t2_sb[p])):
                tp = ps_fin.tile([128, 32], f32, tag="fin", name=f"tp{p}{h}{it}",
                                 padded_shape=[128, 512])
                nc.tensor.transpose(tp[:, :],
                                    o_acc[p][:, h * 128:(h + 1) * 128],
                                    ident_t[0:32, 0:32])
                nc.vector.tensor_copy(t_sb[:, :], tp[:, :])
            # Wo for the pair's 2 batches: accumulate over the two W.T halves
            wo = ps_fin.tile([64, 2 * K], f32, tag="fin", name=f"wo{p}_{it}",
                             padded_shape=[64, 512])
            for h2 in range(2):
                for hb in range(2):
                    nc.tensor.matmul(
                        wo[:, hb * K:(hb + 1) * K],
                        lhsT=wt_t[:, h2 * D:(h2 + 1) * D],
                        rhs=(t1_sb[p], t2_sb[p])[h2][:, hb * K:(hb + 1) * K],
                        start=(h2 == 0 and hb == 0),
                        stop=(h2 == 1 and hb == 1))
            for hb in range(2):
                nc.vector.tensor_copy(
                    wop[p][64 * hb:64 * hb + 64, 16 * hb:16 * hb + 16],
                    wo[:, hb * K:(hb + 1) * K])

        for rep in range(reps):
            if "nodma" not in ablate:
                for p in range(NP):
                    for q in range(ntile):
                        nc.sync.dma_start(out=un_t[p][q][:, :], in_=un_d[p, q])
                    for q in range(ntile):
                        nc.sync.dma_start(out=ut_t[p][q][:, :], in_=ut_d[p, q])
            elif rep == 0:
                for p in range(NP):
                    for q in range(ntile):
                        nc.vector.memset(un_t[p][q][:, 0:2], 0.0)
                        nc.vector.memset(ut_t[p][q][:, 0:2], 0.0)
            if "nocompute" not in ablate:
                for p in range(NP):
                    for it in range(ROUTINGS):
                        gt = pass_pair(p, it)
                        fin_pair(p, it, gt)
            else:
                for p in range(NP):
                    nc.vector.memset(o_fin[p][:, :], 0.0)
                    nc.sync.dma_start(out=out_d[32 * p:32 * p + 32, :],
                                      in_=o_fin[p][:, :])
            if rep < reps - 1:
                tc.strict_bb_all_engine_barrier()

    nc.compile()
    return nc


def host_inputs(u_shard, W):
    """Per-core DRAM inputs from an (8, N, 64) f32 batch shard."""
    n = u_shard.shape[1]
    tcols = min(TCOLS, n)
    ntile = n // tcols
    cpt = tcols // CHUNK
    # ut tile q of pair p: [(h,e), n-slice]  (e-on-partitions, fp8)
    ut = np.ascontiguousarray(
        u_shard.reshape(NP, 2, ntile, tcols, D)
        .transpose(0, 2, 1, 4, 3).reshape(NP, ntile, 128, tcols)
    ).astype(U8_NP)
    # un tile q of pair p: [n-within-chunk, (chunk-rel, h, e)]
    un = np.ascontiguousarray(
        u_shard.reshape(NP, 2, ntile, cpt, CHUNK, D)
        .transpose(0, 2, 4, 3, 1, 5).reshape(NP, ntile, 128, tcols)
    ).astype(U_NP)
    return {"ut": ut, "un": un}


def host_consts(W):
    Wf = np.asarray(W, np.float32)
    wt = np.ascontiguousarray(Wf.T.reshape(2, 128, D)).astype(U_NP)
    wsb = np.ascontiguousarray(np.concatenate([Wf, Wf], 0)).astype(U_NP)
    base = np.kron(np.eye(K, dtype=np.float32), np.ones((1, DCAP), np.float32))
    mask = np.ascontiguousarray(np.tile(base, (NB, 1)))
    ident = np.eye(128, dtype=np.float32)
    return {"wt": wt, "wsb": wsb, "mask": mask, "ident": ident}


def extract_output(res_out):
    """(128, 256) masked f32 -> (8, 16, 16) squashed capsule outputs."""
    ar = np.arange(K)
    return res_out.reshape(NB, K, K, DCAP)[:, ar, ar, :]


_PROG_CACHE = {}


def _get_prog(n=N_FULL, reps=1):
    key = (n, reps)
    if key not in _PROG_CACHE:
        _PROG_CACHE[key] = build_program(n, reps)
    return _PROG_CACHE[key]


def kernel(u_vecs, W):
    u = np.ascontiguousarray(np.asarray(u_vecs, np.float32))
    assert u.shape == (B, N_FULL, D)
    nc = _get_prog()
    consts = host_consts(W)
    in_maps = [dict(consts, **host_inputs(u[c * NB:(c + 1) * NB], W))
               for c in range(NCORES)]
    res = run_bass_kernel_spmd(nc, in_maps, core_ids=list(range(NCORES)))
    return np.concatenate(
        [extract_output(res.results[c]["out"]) for c in range(NCORES)], axis=0
    ).astype(np.float32)
